# revision 85
# baseline (speedup 1.0000x reference)
"""Trainium2 Bass kernel for AngelLoss (center loss + angular loss).

loss = 0.5*sum((feat - centers[y])^2)/B
     + sum_offdiag((c_i.c_j/(|c_i||c_j|) - ct)^2) / (0.5*C*(C-1))

Sharding (8 NeuronCores, data-parallel over batch):
  - center term, gather-free:  sum||f||^2 - 2*sum_c c_c.S_c + sum_c n_c||c_c||^2
    where S_c = sum of feat rows with label c.  Host buckets each core's
    8192 rows into 8 class-banks (125 classes each, greedy-balanced).
    Banks are processed in pairs: one [128, 18, 512] fp8 feat tile per
    pair, slot order [A0-6, B0-6, A7-8, B7-8] so the ScalarE square
    covers slots 0:14 in one instruction and DVE squares slots 14:18 in
    one multiply+reduce, while every DoubleRow matmul slice (2 adjacent
    slots) stays contiguous.  S for the pair accumulates in a 2-bank
    PSUM tile via fp8 onehot^T @ feat matmuls and drains with a DVE
    multiply + free-axis reduce against the resident fp8 centers.
  - angular term via the Frobenius identity (N = row-normalized centers):
      sum_ij (sim-ct)^2 = ||N^T N||_F^2 - 2ct ||sum_i N_i||^2 + C^2 ct^2
    computed redundantly on every core from the resident fp8 normalized
    table with 2x16 accumulating DoubleRow matmuls.
  - per-core [1,32] partial sums are combined on the host.
"""

from contextlib import ExitStack

import ml_dtypes
import numpy as np

import concourse.bass as bass
import concourse.tile as tile
from concourse import bacc, mybir
from concourse.bass import ds, ts
from concourse.bass_utils import run_bass_kernel_spmd

N_CORES = 8
B, C, D = 65536, 1000, 512
BS = B // N_CORES  # 8192 rows per core
NB = 8  # class banks
CPB = C // NB  # 125 classes per bank
SUB = 9  # 128-row subtiles per bank (1152 slots >= ~1024+slack rows)
PSLOT = 2 * SUB  # 18 slots per bank-pair tile
PGROUP = 128 * PSLOT  # 2304 rows per pair block
PR = 4 * PGROUP  # 9216 padded rows per core
NSCP = 14  # slots per pair squared on ScalarE; the rest (4) go to DVE
# slot order within a pair tile: A0-6, B0-6, A7-8, B7-8
SLOT_A = [0, 1, 2, 3, 4, 5, 6, 14, 15]
SLOT_B = [7, 8, 9, 10, 11, 12, 13, 16, 17]

# ct = 2*radius(C-1)^2 - 1 from the reference, evaluated in f64, cast f32.
CT = float(np.float32(-0.0010010010010047532))

_F32 = mybir.dt.float32
_BF16 = mybir.dt.bfloat16
_FP8 = mybir.dt.float8e4

_NC_CACHE = {}


def _scatter(nc, st, oht, ft, base, slots, bankslot):
    """5 matmuls (4x DoubleRow + 1 plain) accumulating one bank's S."""
    pairs = [(slots[0], slots[1]), (slots[2], slots[3]), (slots[4], slots[5]),
             (slots[7], slots[8])]
    for i, (s0, s1) in enumerate(pairs):
        assert s1 == s0 + 1
        nc.tensor.matmul(
            st[:, bankslot, :],
            oht[:, ds(base + s0, 2), :],
            ft[:, ds(s0, 2), :],
            start=(i == 0),
            stop=False,
            perf_mode=mybir.MatmulPerfMode.DoubleRow,
        )
    nc.tensor.matmul(
        st[:, bankslot, :],
        oht[:, base + slots[6], :],
        ft[:, slots[6], :],
        start=False,
        stop=True,
    )


def _build_body(ctx, tc, feat, cnt, oh, cbf, nbf, out):
    nc = tc.nc
    AF = mybir.ActivationFunctionType

    const = ctx.enter_context(tc.tile_pool(name="const", bufs=1))
    pnrm = ctx.enter_context(tc.tile_pool(name="nrm", bufs=2))
    pfeat = ctx.enter_context(tc.tile_pool(name="feat", bufs=2))
    psq = ctx.enter_context(tc.tile_pool(name="sq", bufs=2))
    pdscr = ctx.enter_context(tc.tile_pool(name="dscr", bufs=2))
    pG = ctx.enter_context(tc.tile_pool(name="G", bufs=1, space="PSUM"))
    pS = ctx.enter_context(tc.tile_pool(name="S", bufs=2, space="PSUM"))

    oht = const.tile([128, 4 * PSLOT, 128], _FP8)
    ct_all = const.tile([128, NB, D], _FP8)
    nt_all = const.tile([128, NB, D], _FP8)
    cntt = const.tile([128, 16], _F32)

    ones = const.tile([128, 1], _F32)
    nc.vector.memset(ones[:], 1.0)
    onesp2 = const.tile([128, 2, 1], _FP8)
    nc.vector.memset(onesp2[:], 1.0)
    # staging cols: 0-4 scalar sum(f^2); 8 counts.|c|^2; 9,15 ||G||^2
    # halves; 10-12,24,25 +cross (host applies -2); 14 ||colsum||^2;
    # 16-19 vector sum(f^2).
    staging = const.tile([128, 32], _F32)
    nc.vector.memset(staging[:], 0.0)
    # dummy square so the Square act-table prefetches before the ft0 wait
    warm = pnrm.tile([1, 1], _F32, tag="warm")
    nc.scalar.activation(warm[0:1, :], ones[0:1, :], AF.Square)

    # all loads ride the sync ring, ordered by need-time: the opening
    # feat chunks first (nothing races them), then the first onehot
    # block, the rest of the feat pairs, and the tables.  gpsimd and the
    # scalar stream run no DMA at all.
    fts = []
    for p in range(2):
        ftp = pfeat.tile([128, PSLOT, D], _FP8, tag="ft", name=f"ftp{p}")
        src = feat[ds(p * PGROUP, PGROUP), :].rearrange("(q s) d -> q s d", q=128)
        if p == 0:
            # split so the first ScalarE square can start on slots 0:3
            nc.sync.dma_start(ftp[:, :3, :], src[:, :3, :])
            nc.sync.dma_start(ftp[:, ds(3, 11), :], src[:, ds(3, 11), :])
            nc.sync.dma_start(ftp[:, ds(14, 4), :], src[:, ds(14, 4), :])
            nc.sync.dma_start(oht[:, ds(0, PSLOT), :], oh[:, ds(0, PSLOT), :])
        else:
            nc.sync.dma_start(ftp[:], src[:, :, :])
        fts.append(ftp)
    nc.sync.dma_start(oht[:, ds(PSLOT, PSLOT), :], oh[:, ds(PSLOT, PSLOT), :])
    nc.sync.dma_start(ct_all[:], cbf.rearrange("(p s) d -> p s d", p=128))
    nc.sync.dma_start(nt_all[:], nbf.rearrange("(p s) d -> p s d", p=128))

    # --- center-loss main loop over bank pairs ---
    for p in range(4):
        if p < 2:
            ft = fts[p]
        else:
            ft = pfeat.tile([128, PSLOT, D], _FP8, tag="ft")
            nc.sync.dma_start(
                ft[:],
                feat[ds(p * PGROUP, PGROUP), :].rearrange("(q s) d -> q s d", q=128),
            )
        if p == 1:
            nc.sync.dma_start(
                oht[:, ds(2 * PSLOT, PSLOT), :], oh[:, ds(2 * PSLOT, PSLOT), :]
            )
            nc.sync.dma_start(cntt[:], cnt[:, :])
        elif p == 2:
            nc.sync.dma_start(
                oht[:, ds(3 * PSLOT, PSLOT), :], oh[:, ds(3 * PSLOT, PSLOT), :]
            )
        if p == 3:
            # consume the Gram + colsum banks before the last pair's
            # square so they sit in mid-stream slack, not on the tail
            gsq = pnrm.tile([128, 2, D], _F32, tag="gsq")
            nc.scalar.activation(
                gsq[:], Gt[:], AF.Square, accum_out=staging[:, 9:10]
            )
            css = pnrm.tile([1, D], _F32, tag="css")
            nc.scalar.activation(
                css[0:1, :], csf[0:1, :], AF.Square, accum_out=staging[0:1, 14:15]
            )
        st = pS.tile([128, 2, D], _F32, tag="S")
        sqs = psq.tile([128, NSCP, D], _FP8, tag="sqs")
        if p == 0:
            nc.scalar.activation(
                sqs[:, :3, :], ft[:, :3, :], AF.Square,
                accum_out=staging[:, 0:1],
            )
            nc.scalar.activation(
                sqs[:, 3:NSCP, :], ft[:, ds(3, NSCP - 3), :], AF.Square,
                accum_out=staging[:, 1:2],
            )
        else:
            nc.scalar.activation(
                sqs[:], ft[:, :NSCP, :], AF.Square,
                accum_out=staging[:, 1 + p : 2 + p],
            )
        sqv = psq.tile([128, PSLOT - NSCP, D], _FP8, tag="sqv")
        nc.vector.tensor_tensor(
            out=sqv[:],
            in0=ft[:, ds(NSCP, PSLOT - NSCP), :],
            in1=ft[:, ds(NSCP, PSLOT - NSCP), :],
            op=mybir.AluOpType.mult,
        )
        nc.vector.tensor_reduce(
            out=staging[:, 16 + p : 17 + p],
            in_=sqv[:],
            axis=mybir.AxisListType.XY,
            op=mybir.AluOpType.add,
        )
        _scatter(nc, st, oht, ft, p * PSLOT, SLOT_A, 0)
        _scatter(nc, st, oht, ft, p * PSLOT, SLOT_B, 1)
        dscr = pdscr.tile([CPB, 2, D], _F32, tag="dscr")
        nc.vector.tensor_tensor(
            out=dscr[:],
            in0=st[:CPB, :, :],
            in1=ct_all[:CPB, ds(2 * p, 2), :],
            op=mybir.AluOpType.mult,
        )
        nc.vector.tensor_reduce(
            out=staging[:CPB, 10 + p : 11 + p],
            in_=dscr[:],
            axis=mybir.AxisListType.XY,
            op=mybir.AluOpType.add,
        )
        if p == 1:
            # angular Gram, one 2-bank pass: rows [0:256) of each core's
            # (host-rotated) normalized table; cores 0 and 1 jointly
            # cover all 512 rows of the symmetric Gram, other cores get
            # zero tables
            Gt = pG.tile([128, 2, D], _F32, tag="G")
            for jp in range(0, NB, 2):
                for kx in range(2):
                    nc.tensor.matmul(
                        Gt[:, kx, :],
                        nt_all[:CPB, ds(jp, 2), ts(kx, 128)],
                        nt_all[:CPB, ds(jp, 2), :],
                        start=(jp == 0),
                        stop=(jp == NB - 2),
                        perf_mode=mybir.MatmulPerfMode.DoubleRow,
                    )
        if p == 2:
            # colsum of the normalized table on the idle cs PSUM bank
            csf = pG.tile([1, D], _F32, tag="cs")
            for j in range(NB):
                nc.tensor.matmul(
                    csf[0:1, :],
                    onesp2[:CPB, 0, :],
                    nt_all[:CPB, j, :],
                    start=(j == 0),
                    stop=(j == NB - 1),
                )
        if p == 3:
            # counts . |c|^2 (norms^2 host-packed beside the counts)
            cscr = pnrm.tile([CPB, NB], _F32, tag="cscr")
            nc.vector.tensor_tensor(
                out=cscr[:],
                in0=cntt[:CPB, 0:NB],
                in1=cntt[:CPB, NB:16],
                op=mybir.AluOpType.mult,
            )
            nc.vector.tensor_reduce(
                out=staging[:CPB, 8:9],
                in_=cscr[:],
                axis=mybir.AxisListType.X,
                op=mybir.AluOpType.add,
            )

    pf = pG.tile([1, 32], _F32, tag="cs")
    nc.tensor.matmul(pf[:], ones[:], staging[:], start=True, stop=True)
    osb = const.tile([1, 32], _F32)
    nc.vector.tensor_copy(osb[:], pf[:])
    nc.sync.dma_start(out[:, :], osb[:, :])


def build():
    if "nc" in _NC_CACHE:
        return _NC_CACHE["nc"]
    nc = bacc.Bacc(
        "TRN2",
        target_bir_lowering=False,
        debug=False,
        enable_asserts=False,
        num_devices=N_CORES,
    )
    feat = nc.dram_tensor("feat", [PR, D], _FP8, kind="ExternalInput").ap()
    cnt = nc.dram_tensor("cnt", [128, 16], _F32, kind="ExternalInput").ap()
    oh = nc.dram_tensor("oh", [128, 4 * PSLOT, 128], _FP8, kind="ExternalInput").ap()
    cbf = nc.dram_tensor("ctab", [128 * NB, D], _FP8, kind="ExternalInput").ap()
    nbf = nc.dram_tensor("ntab", [128 * NB, D], _FP8, kind="ExternalInput").ap()
    out = nc.dram_tensor("out", [1, 32], _F32, kind="ExternalOutput").ap()
    with tile.TileContext(nc) as tc, ExitStack() as ctx:
        _build_body(ctx, tc, feat, cnt, oh, cbf, nbf, out)
    nc.compile()
    _NC_CACHE["nc"] = nc
    return nc


def _bank_assignment(y):
    """Greedy-balanced partition of the C classes into NB banks of CPB each."""
    counts = np.bincount(y, minlength=C)
    order = np.argsort(-counts, kind="stable")
    bank_tot = np.zeros(NB, dtype=np.int64)
    bank_n = np.zeros(NB, dtype=np.int64)
    bankclasses = np.zeros((NB, CPB), dtype=np.int64)
    cls_bank = np.zeros(C, dtype=np.int64)
    cls_pos = np.zeros(C, dtype=np.int64)
    for c in order:
        open_banks = np.flatnonzero(bank_n < CPB)
        j = open_banks[np.argmin(bank_tot[open_banks])]
        bankclasses[j, bank_n[j]] = c
        cls_bank[c] = j
        cls_pos[c] = bank_n[j]
        bank_n[j] += 1
        bank_tot[j] += counts[c]
    assert bank_tot.max() <= 128 * SUB, f"bank overflow: {bank_tot.max()}"
    return bankclasses, cls_bank, cls_pos, counts


def make_in_maps(y, feat, centers):
    feat = np.ascontiguousarray(feat, dtype=np.float32)
    centers = np.ascontiguousarray(centers, dtype=np.float32)
    y = np.asarray(y).astype(np.int64)
    norm2 = np.sum(centers.astype(np.float64) ** 2, axis=1, keepdims=True)
    ncenters = (centers / np.sqrt(norm2)).astype(ml_dtypes.float8_e4m3)
    slot_of = np.zeros((2, SUB), dtype=np.int64)
    slot_of[0] = SLOT_A
    slot_of[1] = SLOT_B
    in_maps = []
    for i in range(N_CORES):
        ys = y[i * BS : (i + 1) * BS]
        fs = feat[i * BS : (i + 1) * BS]
        bankclasses, cls_bank, cls_pos, counts = _bank_assignment(ys)

        # bank-major padded tables: dram row r (r%128 < 125) = class
        # bankclasses[r // 128][r % 128]
        ctab = np.zeros((128 * NB, D), dtype=ml_dtypes.float8_e4m3)
        ntab = np.zeros((128 * NB, D), dtype=ml_dtypes.float8_e4m3)
        rr = np.arange(128 * NB)
        vr = rr % 128 < CPB
        src = bankclasses[rr[vr] // 128, rr[vr] % 128]
        ctab[vr] = centers[src].astype(ml_dtypes.float8_e4m3)
        if i == 0:
            ntab[vr] = ncenters[src]
        elif i == 1:
            # D-rotated so this core's Gram pass covers rows [256:512)
            ntab[vr] = ncenters[src][:, (np.arange(D) + 256) % D]
        # cores 2-7 keep zero tables: their Gram/colsum outputs are zero

        # bucket rows by bank; bank q's i-th row sits at pair p=q//2,
        # partition i//9, slot slot_of[q%2][i%9]; padded row index is
        # p*PGROUP + 18*part + slot
        row_bank = cls_bank[ys]
        grp_order = np.argsort(row_bank, kind="stable")
        n_per = np.bincount(row_bank, minlength=NB)
        starts = np.zeros(NB + 1, dtype=np.int64)
        starts[1:] = np.cumsum(n_per)

        featp = np.zeros((PR, D), dtype=ml_dtypes.float8_e4m3)
        oh = np.zeros((128, 4 * PSLOT, 128), dtype=ml_dtypes.float8_e4m3)
        for q in range(NB):
            rows = grp_order[starts[q] : starts[q + 1]]
            idx = np.arange(len(rows))
            part = idx // SUB
            slot = slot_of[q % 2][idx % SUB]
            prow = (q // 2) * PGROUP + 18 * part + slot
            featp[prow] = fs[rows].astype(ml_dtypes.float8_e4m3)
            oh[part, (q // 2) * PSLOT + slot, cls_pos[ys[rows]]] = 1.0

        cnt_pb = np.zeros((128, 16), dtype=np.float32)
        cnt_pb[cls_pos, cls_bank] = counts
        cnt_pb[cls_pos, NB + cls_bank] = norm2[:, 0]

        in_maps.append(
            {
                "feat": featp,
                "cnt": cnt_pb,
                "oh": oh,
                "ctab": ctab,
                "ntab": ntab,
            }
        )
    return in_maps


def combine(outs):
    """outs: list of 8 [1,32] f32 arrays -> scalar loss (np.float32)."""
    cen = 0.0
    for o in outs:
        o = np.asarray(o, dtype=np.float64)
        cen += o[0, 0:9].sum() + o[0, 16:24].sum() - 2.0 * o[0, 10:14].sum()
    o0 = np.asarray(outs[0], dtype=np.float64)
    o1 = np.asarray(outs[1], dtype=np.float64)
    gsq, ssq = o0[0, 9] + o1[0, 9], o0[0, 14]
    ang = gsq - 2.0 * CT * ssq + C * C * CT * CT - C * (1.0 - CT) ** 2
    loss = 0.5 * cen / B + ang / (0.5 * C * (C - 1))
    return np.float32(loss)


def kernel(y, feat, centers):
    nc = build()
    in_maps = make_in_maps(y, feat, centers)
    res = run_bass_kernel_spmd(nc, in_maps, core_ids=list(range(N_CORES)))
    return combine([res.results[i]["out"] for i in range(N_CORES)])


# revision 86
# speedup vs baseline: 1.1411x; 1.1411x over previous
"""Trainium2 Bass kernel for AngelLoss (center loss + angular loss).

loss = 0.5*sum((feat - centers[y])^2)/B
     + sum_offdiag((c_i.c_j/(|c_i||c_j|) - ct)^2) / (0.5*C*(C-1))

Sharding (8 NeuronCores, data-parallel over batch):
  - center term, gather-free:  sum||f||^2 - 2*sum_c c_c.S_c + sum_c n_c||c_c||^2
    where S_c = sum of feat rows with label c.  Host buckets each core's
    8192 rows into 8 class-banks (125 classes each, greedy-balanced).
    Banks are processed in pairs: one [128, 18, 512] fp8 feat tile per
    pair, slot order [A0-6, B0-6, A7-8, B7-8] so the ScalarE square
    covers slots 0:14 in one instruction and DVE squares slots 14:18 in
    one multiply+reduce, while every DoubleRow matmul slice (2 adjacent
    slots) stays contiguous.  S for the pair accumulates in a 2-bank
    PSUM tile via fp8 onehot^T @ feat matmuls and drains with a DVE
    multiply + free-axis reduce against the resident fp8 centers.
  - angular term via the Frobenius identity (N = row-normalized centers):
      sum_ij (sim-ct)^2 = ||N^T N||_F^2 - 2ct ||sum_i N_i||^2 + C^2 ct^2
    computed redundantly on every core from the resident fp8 normalized
    table with 2x16 accumulating DoubleRow matmuls.
  - per-core [1,32] partial sums are combined on the host.
"""

from contextlib import ExitStack

import ml_dtypes
import numpy as np

import concourse.bass as bass
import concourse.tile as tile
from concourse import bacc, mybir
from concourse.bass import ds, ts
from concourse.bass_utils import run_bass_kernel_spmd

N_CORES = 8
B, C, D = 65536, 1000, 512
BS = B // N_CORES  # 8192 rows per core
NB = 8  # class banks
CPB = C // NB  # 125 classes per bank
SUB = 9  # 128-row subtiles per bank (1152 slots >= ~1024+slack rows)
PSLOT = 2 * SUB  # 18 slots per bank-pair tile
PGROUP = 128 * PSLOT  # 2304 rows per pair block
PR = 4 * PGROUP  # 9216 padded rows per core
NSCP = 14  # slots per pair squared on ScalarE; the rest (4) go to DVE
# slot order within a pair tile: A0-6, B0-6, A7-8, B7-8
SLOT_A = [0, 1, 2, 3, 4, 5, 6, 14, 15]
SLOT_B = [7, 8, 9, 10, 11, 12, 13, 16, 17]

# ct = 2*radius(C-1)^2 - 1 from the reference, evaluated in f64, cast f32.
CT = float(np.float32(-0.0010010010010047532))

_F32 = mybir.dt.float32
_BF16 = mybir.dt.bfloat16
_FP8 = mybir.dt.float8e4

_NC_CACHE = {}


def _scatter(nc, st, oht, ft, base, slots, bankslot):
    """5 matmuls (4x DoubleRow + 1 plain) accumulating one bank's S."""
    pairs = [(slots[0], slots[1]), (slots[2], slots[3]), (slots[4], slots[5]),
             (slots[7], slots[8])]
    for i, (s0, s1) in enumerate(pairs):
        assert s1 == s0 + 1
        nc.tensor.matmul(
            st[:, bankslot, :],
            oht[:, ds(base + s0, 2), :],
            ft[:, ds(s0, 2), :],
            start=(i == 0),
            stop=False,
            perf_mode=mybir.MatmulPerfMode.DoubleRow,
        )
    nc.tensor.matmul(
        st[:, bankslot, :],
        oht[:, base + slots[6], :],
        ft[:, slots[6], :],
        start=False,
        stop=True,
    )


def _build_body(ctx, tc, feat, cnt, oh, cbf, nbf, out):
    nc = tc.nc
    AF = mybir.ActivationFunctionType

    const = ctx.enter_context(tc.tile_pool(name="const", bufs=1))
    pnrm = ctx.enter_context(tc.tile_pool(name="nrm", bufs=2))
    pfeat = ctx.enter_context(tc.tile_pool(name="feat", bufs=2))
    psq = ctx.enter_context(tc.tile_pool(name="sq", bufs=2))
    pdscr = ctx.enter_context(tc.tile_pool(name="dscr", bufs=2))
    pG = ctx.enter_context(tc.tile_pool(name="G", bufs=1, space="PSUM"))
    pS = ctx.enter_context(tc.tile_pool(name="S", bufs=2, space="PSUM"))

    oht = const.tile([128, 4 * PSLOT, 128], _FP8)
    ct_all = const.tile([128, NB, D], _FP8)
    nt_all = const.tile([128, NB, D], _FP8)
    cntt = const.tile([128, 16], _F32)

    ones = const.tile([128, 1], _F32)
    nc.vector.memset(ones[:], 1.0)
    onesp2 = const.tile([128, 2, 1], _FP8)
    nc.vector.memset(onesp2[:], 1.0)
    # staging cols: 0-4 scalar sum(f^2); 8 counts.|c|^2; 9,15 ||G||^2
    # halves; 10-12,24,25 +cross (host applies -2); 14 ||colsum||^2;
    # 16-19 vector sum(f^2).
    staging = const.tile([128, 32], _F32)
    nc.vector.memset(staging[:], 0.0)
    # dummy square so the Square act-table prefetches before the ft0 wait
    warm = pnrm.tile([1, 1], _F32, tag="warm")
    nc.scalar.activation(warm[0:1, :], ones[0:1, :], AF.Square)

    # all loads ride the sync ring, ordered by need-time: the opening
    # feat chunks first (nothing races them), then the first onehot
    # block, the rest of the feat pairs, and the tables.  gpsimd and the
    # scalar stream run no DMA at all.
    fts = []
    for p in range(2):
        ftp = pfeat.tile([128, PSLOT, D], _FP8, tag="ft", name=f"ftp{p}")
        src = feat[ds(p * PGROUP, PGROUP), :].rearrange("(q s) d -> q s d", q=128)
        if p == 0:
            # split so the first ScalarE square can start on slots 0:3
            nc.sync.dma_start(ftp[:, :3, :], src[:, :3, :])
            nc.sync.dma_start(ftp[:, ds(3, 11), :], src[:, ds(3, 11), :])
            nc.sync.dma_start(ftp[:, ds(14, 4), :], src[:, ds(14, 4), :])
            nc.sync.dma_start(oht[:, ds(0, PSLOT), :], oh[:, ds(0, PSLOT), :])
        else:
            nc.sync.dma_start(ftp[:], src[:, :, :])
        fts.append(ftp)
    nc.sync.dma_start(oht[:, ds(PSLOT, PSLOT), :], oh[:, ds(PSLOT, PSLOT), :])
    nc.sync.dma_start(ct_all[:], cbf.rearrange("(p s) d -> p s d", p=128))
    nc.sync.dma_start(nt_all[:], nbf.rearrange("(p s) d -> p s d", p=128))

    # --- center-loss main loop over bank pairs ---
    for p in range(4):
        if p < 2:
            ft = fts[p]
        else:
            ft = pfeat.tile([128, PSLOT, D], _FP8, tag="ft")
            nc.sync.dma_start(
                ft[:],
                feat[ds(p * PGROUP, PGROUP), :].rearrange("(q s) d -> q s d", q=128),
            )
        if p == 1:
            nc.sync.dma_start(
                oht[:, ds(2 * PSLOT, PSLOT), :], oh[:, ds(2 * PSLOT, PSLOT), :]
            )
            nc.sync.dma_start(cntt[:], cnt[:, :])
        elif p == 2:
            nc.sync.dma_start(
                oht[:, ds(3 * PSLOT, PSLOT), :], oh[:, ds(3 * PSLOT, PSLOT), :]
            )
        if p == 3:
            # consume the Gram + colsum banks before the last pair's
            # square so they sit in mid-stream slack, not on the tail
            gsq = pnrm.tile([128, 2, D], _F32, tag="gsq")
            nc.scalar.activation(
                gsq[:], Gt[:], AF.Square, accum_out=staging[:, 9:10]
            )
            css = pnrm.tile([1, D], _F32, tag="css")
            nc.scalar.activation(
                css[0:1, :], csf[0:1, :], AF.Square, accum_out=staging[0:1, 14:15]
            )
        st = pS.tile([128, 2, D], _F32, tag="S")
        sqs = psq.tile([128, NSCP, D], _FP8, tag="sqs")
        if p == 0:
            nc.scalar.activation(
                sqs[:, :3, :], ft[:, :3, :], AF.Square,
                accum_out=staging[:, 0:1],
            )
            nc.scalar.activation(
                sqs[:, 3:NSCP, :], ft[:, ds(3, NSCP - 3), :], AF.Square,
                accum_out=staging[:, 1:2],
            )
        else:
            nc.scalar.activation(
                sqs[:], ft[:, :NSCP, :], AF.Square,
                accum_out=staging[:, 1 + p : 2 + p],
            )
        sqv = psq.tile([128, PSLOT - NSCP, D], _FP8, tag="sqv")
        nc.vector.tensor_tensor(
            out=sqv[:],
            in0=ft[:, ds(NSCP, PSLOT - NSCP), :],
            in1=ft[:, ds(NSCP, PSLOT - NSCP), :],
            op=mybir.AluOpType.mult,
        )
        nc.vector.tensor_reduce(
            out=staging[:, 16 + p : 17 + p],
            in_=sqv[:],
            axis=mybir.AxisListType.XY,
            op=mybir.AluOpType.add,
        )
        _scatter(nc, st, oht, ft, p * PSLOT, SLOT_A, 0)
        _scatter(nc, st, oht, ft, p * PSLOT, SLOT_B, 1)
        dscr = pdscr.tile([CPB, 2, D], _F32, tag="dscr")
        nc.vector.tensor_tensor(
            out=dscr[:],
            in0=st[:CPB, :, :],
            in1=ct_all[:CPB, ds(2 * p, 2), :],
            op=mybir.AluOpType.mult,
        )
        nc.vector.tensor_reduce(
            out=staging[:CPB, 10 + p : 11 + p],
            in_=dscr[:],
            axis=mybir.AxisListType.XY,
            op=mybir.AluOpType.add,
        )
        if p == 1:
            # angular Gram, one 2-bank pass: rows [0:256) of each core's
            # (host-rotated) normalized table; cores 0 and 1 jointly
            # cover all 512 rows of the symmetric Gram, other cores get
            # zero tables
            Gt = pG.tile([128, 2, D], _F32, tag="G")
            for jp in range(0, NB, 2):
                for kx in range(2):
                    nc.tensor.matmul(
                        Gt[:, kx, :],
                        nt_all[:CPB, ds(jp, 2), ts(kx, 128)],
                        nt_all[:CPB, ds(jp, 2), :],
                        start=(jp == 0),
                        stop=(jp == NB - 2),
                        perf_mode=mybir.MatmulPerfMode.DoubleRow,
                    )
        if p == 2:
            # colsum of the normalized table on the idle cs PSUM bank
            csf = pG.tile([1, D], _F32, tag="cs")
            for j in range(NB):
                nc.tensor.matmul(
                    csf[0:1, :],
                    onesp2[:CPB, 0, :],
                    nt_all[:CPB, j, :],
                    start=(j == 0),
                    stop=(j == NB - 1),
                )
        if p == 2:
            # counts . |c|^2 (norms^2 host-packed beside the counts);
            # runs in DVE mid-stream slack, off the final-collapse tail
            cscr = pnrm.tile([CPB, NB], _F32, tag="cscr")
            nc.vector.tensor_tensor(
                out=cscr[:],
                in0=cntt[:CPB, 0:NB],
                in1=cntt[:CPB, NB:16],
                op=mybir.AluOpType.mult,
            )
            nc.vector.tensor_reduce(
                out=staging[:CPB, 8:9],
                in_=cscr[:],
                axis=mybir.AxisListType.X,
                op=mybir.AluOpType.add,
            )

    pf = pG.tile([1, 32], _F32, tag="cs")
    nc.tensor.matmul(pf[:], ones[:], staging[:], start=True, stop=True)
    osb = const.tile([1, 32], _F32)
    nc.vector.tensor_copy(osb[:], pf[:])
    nc.sync.dma_start(out[:, :], osb[:, :])


def build():
    if "nc" in _NC_CACHE:
        return _NC_CACHE["nc"]
    nc = bacc.Bacc(
        "TRN2",
        target_bir_lowering=False,
        debug=False,
        enable_asserts=False,
        num_devices=N_CORES,
    )
    feat = nc.dram_tensor("feat", [PR, D], _FP8, kind="ExternalInput").ap()
    cnt = nc.dram_tensor("cnt", [128, 16], _F32, kind="ExternalInput").ap()
    oh = nc.dram_tensor("oh", [128, 4 * PSLOT, 128], _FP8, kind="ExternalInput").ap()
    cbf = nc.dram_tensor("ctab", [128 * NB, D], _FP8, kind="ExternalInput").ap()
    nbf = nc.dram_tensor("ntab", [128 * NB, D], _FP8, kind="ExternalInput").ap()
    out = nc.dram_tensor("out", [1, 32], _F32, kind="ExternalOutput").ap()
    with tile.TileContext(nc) as tc, ExitStack() as ctx:
        _build_body(ctx, tc, feat, cnt, oh, cbf, nbf, out)
    nc.compile()
    _NC_CACHE["nc"] = nc
    return nc


def _bank_assignment(y):
    """Greedy-balanced partition of the C classes into NB banks of CPB each."""
    counts = np.bincount(y, minlength=C)
    order = np.argsort(-counts, kind="stable")
    bank_tot = np.zeros(NB, dtype=np.int64)
    bank_n = np.zeros(NB, dtype=np.int64)
    bankclasses = np.zeros((NB, CPB), dtype=np.int64)
    cls_bank = np.zeros(C, dtype=np.int64)
    cls_pos = np.zeros(C, dtype=np.int64)
    for c in order:
        open_banks = np.flatnonzero(bank_n < CPB)
        j = open_banks[np.argmin(bank_tot[open_banks])]
        bankclasses[j, bank_n[j]] = c
        cls_bank[c] = j
        cls_pos[c] = bank_n[j]
        bank_n[j] += 1
        bank_tot[j] += counts[c]
    assert bank_tot.max() <= 128 * SUB, f"bank overflow: {bank_tot.max()}"
    return bankclasses, cls_bank, cls_pos, counts


def make_in_maps(y, feat, centers):
    feat = np.ascontiguousarray(feat, dtype=np.float32)
    centers = np.ascontiguousarray(centers, dtype=np.float32)
    y = np.asarray(y).astype(np.int64)
    norm2 = np.sum(centers.astype(np.float64) ** 2, axis=1, keepdims=True)
    ncenters = (centers / np.sqrt(norm2)).astype(ml_dtypes.float8_e4m3)
    slot_of = np.zeros((2, SUB), dtype=np.int64)
    slot_of[0] = SLOT_A
    slot_of[1] = SLOT_B
    in_maps = []
    for i in range(N_CORES):
        ys = y[i * BS : (i + 1) * BS]
        fs = feat[i * BS : (i + 1) * BS]
        bankclasses, cls_bank, cls_pos, counts = _bank_assignment(ys)

        # bank-major padded tables: dram row r (r%128 < 125) = class
        # bankclasses[r // 128][r % 128]
        ctab = np.zeros((128 * NB, D), dtype=ml_dtypes.float8_e4m3)
        ntab = np.zeros((128 * NB, D), dtype=ml_dtypes.float8_e4m3)
        rr = np.arange(128 * NB)
        vr = rr % 128 < CPB
        src = bankclasses[rr[vr] // 128, rr[vr] % 128]
        ctab[vr] = centers[src].astype(ml_dtypes.float8_e4m3)
        if i == 0:
            ntab[vr] = ncenters[src]
        elif i == 1:
            # D-rotated so this core's Gram pass covers rows [256:512)
            ntab[vr] = ncenters[src][:, (np.arange(D) + 256) % D]
        # cores 2-7 keep zero tables: their Gram/colsum outputs are zero

        # bucket rows by bank; bank q's i-th row sits at pair p=q//2,
        # partition i//9, slot slot_of[q%2][i%9]; padded row index is
        # p*PGROUP + 18*part + slot
        row_bank = cls_bank[ys]
        grp_order = np.argsort(row_bank, kind="stable")
        n_per = np.bincount(row_bank, minlength=NB)
        starts = np.zeros(NB + 1, dtype=np.int64)
        starts[1:] = np.cumsum(n_per)

        featp = np.zeros((PR, D), dtype=ml_dtypes.float8_e4m3)
        oh = np.zeros((128, 4 * PSLOT, 128), dtype=ml_dtypes.float8_e4m3)
        for q in range(NB):
            rows = grp_order[starts[q] : starts[q + 1]]
            idx = np.arange(len(rows))
            part = idx // SUB
            slot = slot_of[q % 2][idx % SUB]
            prow = (q // 2) * PGROUP + 18 * part + slot
            featp[prow] = fs[rows].astype(ml_dtypes.float8_e4m3)
            oh[part, (q // 2) * PSLOT + slot, cls_pos[ys[rows]]] = 1.0

        cnt_pb = np.zeros((128, 16), dtype=np.float32)
        cnt_pb[cls_pos, cls_bank] = counts
        cnt_pb[cls_pos, NB + cls_bank] = norm2[:, 0]

        in_maps.append(
            {
                "feat": featp,
                "cnt": cnt_pb,
                "oh": oh,
                "ctab": ctab,
                "ntab": ntab,
            }
        )
    return in_maps


def combine(outs):
    """outs: list of 8 [1,32] f32 arrays -> scalar loss (np.float32)."""
    cen = 0.0
    for o in outs:
        o = np.asarray(o, dtype=np.float64)
        cen += o[0, 0:9].sum() + o[0, 16:24].sum() - 2.0 * o[0, 10:14].sum()
    o0 = np.asarray(outs[0], dtype=np.float64)
    o1 = np.asarray(outs[1], dtype=np.float64)
    gsq, ssq = o0[0, 9] + o1[0, 9], o0[0, 14]
    ang = gsq - 2.0 * CT * ssq + C * C * CT * CT - C * (1.0 - CT) ** 2
    loss = 0.5 * cen / B + ang / (0.5 * C * (C - 1))
    return np.float32(loss)


def kernel(y, feat, centers):
    nc = build()
    in_maps = make_in_maps(y, feat, centers)
    res = run_bass_kernel_spmd(nc, in_maps, core_ids=list(range(N_CORES)))
    return combine([res.results[i]["out"] for i in range(N_CORES)])


# revision 87
# speedup vs baseline: 1.1623x; 1.0186x over previous
"""Trainium2 Bass kernel for AngelLoss (center loss + angular loss).

loss = 0.5*sum((feat - centers[y])^2)/B
     + sum_offdiag((c_i.c_j/(|c_i||c_j|) - ct)^2) / (0.5*C*(C-1))

Sharding (8 NeuronCores, data-parallel over batch):
  - center term, gather-free:  sum||f||^2 - 2*sum_c c_c.S_c + sum_c n_c||c_c||^2
    where S_c = sum of feat rows with label c.  Host buckets each core's
    8192 rows into 8 class-banks (125 classes each, greedy-balanced).
    Banks are processed in pairs: one [128, 18, 512] fp8 feat tile per
    pair, slot order [A0-6, B0-6, A7-8, B7-8] so the ScalarE square
    covers slots 0:14 in one instruction and DVE squares slots 14:18 in
    one multiply+reduce, while every DoubleRow matmul slice (2 adjacent
    slots) stays contiguous.  S for the pair accumulates in a 2-bank
    PSUM tile via fp8 onehot^T @ feat matmuls and drains with a DVE
    multiply + free-axis reduce against the resident fp8 centers.
  - angular term via the Frobenius identity (N = row-normalized centers):
      sum_ij (sim-ct)^2 = ||N^T N||_F^2 - 2ct ||sum_i N_i||^2 + C^2 ct^2
    computed redundantly on every core from the resident fp8 normalized
    table with 2x16 accumulating DoubleRow matmuls.
  - per-core [1,32] partial sums are combined on the host.
"""

from contextlib import ExitStack

import ml_dtypes
import numpy as np

import concourse.bass as bass
import concourse.tile as tile
from concourse import bacc, mybir
from concourse.bass import ds, ts
from concourse.bass_utils import run_bass_kernel_spmd

N_CORES = 8
B, C, D = 65536, 1000, 512
BS = B // N_CORES  # 8192 rows per core
NB = 8  # class banks
CPB = C // NB  # 125 classes per bank
SUB = 9  # 128-row subtiles per bank (1152 slots >= ~1024+slack rows)
PSLOT = 2 * SUB  # 18 slots per bank-pair tile
PGROUP = 128 * PSLOT  # 2304 rows per pair block
PR = 4 * PGROUP  # 9216 padded rows per core
NSCP = 14  # slots per pair squared on ScalarE; the rest (4) go to DVE
# slot order within a pair tile: A0-6, B0-6, A7-8, B7-8
SLOT_A = [0, 1, 2, 3, 4, 5, 6, 14, 15]
SLOT_B = [7, 8, 9, 10, 11, 12, 13, 16, 17]

# ct = 2*radius(C-1)^2 - 1 from the reference, evaluated in f64, cast f32.
CT = float(np.float32(-0.0010010010010047532))

_F32 = mybir.dt.float32
_BF16 = mybir.dt.bfloat16
_FP8 = mybir.dt.float8e4

_NC_CACHE = {}


def _scatter(nc, st, oht, ft, base, slots, bankslot):
    """5 matmuls (4x DoubleRow + 1 plain) accumulating one bank's S."""
    pairs = [(slots[0], slots[1]), (slots[2], slots[3]), (slots[4], slots[5]),
             (slots[7], slots[8])]
    for i, (s0, s1) in enumerate(pairs):
        assert s1 == s0 + 1
        nc.tensor.matmul(
            st[:, bankslot, :],
            oht[:, ds(base + s0, 2), :],
            ft[:, ds(s0, 2), :],
            start=(i == 0),
            stop=False,
            perf_mode=mybir.MatmulPerfMode.DoubleRow,
        )
    nc.tensor.matmul(
        st[:, bankslot, :],
        oht[:, base + slots[6], :],
        ft[:, slots[6], :],
        start=False,
        stop=True,
    )


def _build_body(ctx, tc, feat, cnt, oh, cbf, nbf, out):
    nc = tc.nc
    AF = mybir.ActivationFunctionType

    const = ctx.enter_context(tc.tile_pool(name="const", bufs=1))
    pnrm = ctx.enter_context(tc.tile_pool(name="nrm", bufs=2))
    pfeat = ctx.enter_context(tc.tile_pool(name="feat", bufs=2))
    psq = ctx.enter_context(tc.tile_pool(name="sq", bufs=2))
    pdscr = ctx.enter_context(tc.tile_pool(name="dscr", bufs=2))
    pG = ctx.enter_context(tc.tile_pool(name="G", bufs=1, space="PSUM"))
    pS = ctx.enter_context(tc.tile_pool(name="S", bufs=2, space="PSUM"))

    oht = const.tile([128, 4 * PSLOT, 128], _FP8)
    ct_all = const.tile([128, NB, D], _FP8)
    nt_all = const.tile([128, NB, D], _FP8)
    cntt = const.tile([128, 16], _F32)

    ones = const.tile([128, 1], _F32)
    nc.vector.memset(ones[:], 1.0)
    onesp2 = const.tile([128, 2, 1], _FP8)
    nc.vector.memset(onesp2[:], 1.0)
    # staging cols: 0-4 scalar sum(f^2); 8 counts.|c|^2; 9,15 ||G||^2
    # halves; 10-12,24,25 +cross (host applies -2); 14 ||colsum||^2;
    # 16-19 vector sum(f^2).
    staging = const.tile([128, 32], _F32)
    nc.vector.memset(staging[:], 0.0)
    # dummy square so the Square act-table prefetches before the ft0 wait
    warm = pnrm.tile([1, 1], _F32, tag="warm")
    nc.scalar.activation(warm[0:1, :], ones[0:1, :], AF.Square)

    # all loads ride the sync ring, ordered by need-time: the opening
    # feat chunks first (nothing races them), then the first onehot
    # block, the rest of the feat pairs, and the tables.  gpsimd and the
    # scalar stream run no DMA at all.
    fts = []
    for p in range(2):
        ftp = pfeat.tile([128, PSLOT, D], _FP8, tag="ft", name=f"ftp{p}")
        src = feat[ds(p * PGROUP, PGROUP), :].rearrange("(q s) d -> q s d", q=128)
        if p == 0:
            # split so the first ScalarE square can start on slots 0:3
            nc.sync.dma_start(ftp[:, :3, :], src[:, :3, :])
            nc.sync.dma_start(ftp[:, ds(3, 11), :], src[:, ds(3, 11), :])
            nc.sync.dma_start(ftp[:, ds(14, 4), :], src[:, ds(14, 4), :])
            nc.sync.dma_start(oht[:, ds(0, PSLOT), :], oh[:, ds(0, PSLOT), :])
        else:
            nc.sync.dma_start(ftp[:], src[:, :, :])
        fts.append(ftp)
    nc.sync.dma_start(oht[:, ds(PSLOT, PSLOT), :], oh[:, ds(PSLOT, PSLOT), :])
    nc.sync.dma_start(ct_all[:], cbf.rearrange("(p s) d -> p s d", p=128))
    nc.sync.dma_start(nt_all[:], nbf.rearrange("(p s) d -> p s d", p=128))

    # --- center-loss main loop over bank pairs ---
    for p in range(4):
        if p < 2:
            ft = fts[p]
        else:
            ft = pfeat.tile([128, PSLOT, D], _FP8, tag="ft")
            nc.sync.dma_start(
                ft[:],
                feat[ds(p * PGROUP, PGROUP), :].rearrange("(q s) d -> q s d", q=128),
            )
        if p == 1:
            nc.sync.dma_start(
                oht[:, ds(2 * PSLOT, PSLOT), :], oh[:, ds(2 * PSLOT, PSLOT), :]
            )
            nc.sync.dma_start(cntt[:], cnt[:, :])
        elif p == 2:
            nc.sync.dma_start(
                oht[:, ds(3 * PSLOT, PSLOT), :], oh[:, ds(3 * PSLOT, PSLOT), :]
            )
        if p == 3:
            # consume the Gram + colsum banks before the last pair's
            # square so they sit in mid-stream slack, not on the tail
            gsq = pnrm.tile([128, 2, D], _F32, tag="gsq")
            nc.scalar.activation(
                gsq[:], Gt[:], AF.Square, accum_out=staging[:, 9:10]
            )
            css = pnrm.tile([1, D], _F32, tag="css")
            nc.scalar.activation(
                css[0:1, :], csf[0:1, :], AF.Square, accum_out=staging[0:1, 14:15]
            )
        st = pS.tile([128, 2, D], _F32, tag="S")
        sqs = psq.tile([128, NSCP, D], _FP8, tag="sqs")
        if p == 0:
            nc.scalar.activation(
                sqs[:, :3, :], ft[:, :3, :], AF.Square,
                accum_out=staging[:, 0:1],
            )
            nc.scalar.activation(
                sqs[:, 3:NSCP, :], ft[:, ds(3, NSCP - 3), :], AF.Square,
                accum_out=staging[:, 1:2],
            )
        else:
            nc.scalar.activation(
                sqs[:], ft[:, :NSCP, :], AF.Square,
                accum_out=staging[:, 1 + p : 2 + p],
            )
        sqv = psq.tile([128, PSLOT - NSCP, D], _FP8, tag="sqv")
        nc.vector.tensor_tensor(
            out=sqv[:],
            in0=ft[:, ds(NSCP, PSLOT - NSCP), :],
            in1=ft[:, ds(NSCP, PSLOT - NSCP), :],
            op=mybir.AluOpType.mult,
        )
        nc.vector.tensor_reduce(
            out=staging[:, 16 + p : 17 + p],
            in_=sqv[:],
            axis=mybir.AxisListType.XY,
            op=mybir.AluOpType.add,
        )
        _scatter(nc, st, oht, ft, p * PSLOT, SLOT_A, 0)
        _scatter(nc, st, oht, ft, p * PSLOT, SLOT_B, 1)
        dscr = pdscr.tile([CPB, 2, D], _F32, tag="dscr")
        nc.vector.tensor_tensor(
            out=dscr[:],
            in0=st[:CPB, :, :],
            in1=ct_all[:CPB, ds(2 * p, 2), :],
            op=mybir.AluOpType.mult,
        )
        nc.vector.tensor_reduce(
            out=staging[:CPB, 10 + p : 11 + p],
            in_=dscr[:],
            axis=mybir.AxisListType.XY,
            op=mybir.AluOpType.add,
        )
        if p == 1:
            # angular Gram, one 2-bank pass: rows [0:256) of each core's
            # (host-rotated) normalized table; cores 0 and 1 jointly
            # cover all 512 rows of the symmetric Gram, other cores get
            # zero tables
            Gt = pG.tile([128, 2, D], _F32, tag="G")
            for jp in range(0, NB, 2):
                for kx in range(2):
                    nc.tensor.matmul(
                        Gt[:, kx, :],
                        nt_all[:CPB, ds(jp, 2), ts(kx, 128)],
                        nt_all[:CPB, ds(jp, 2), :],
                        start=(jp == 0),
                        stop=(jp == NB - 2),
                        perf_mode=mybir.MatmulPerfMode.DoubleRow,
                    )
        if p == 2:
            # colsum of the normalized table on the idle cs PSUM bank
            csf = pG.tile([1, D], _F32, tag="cs")
            for j in range(NB):
                nc.tensor.matmul(
                    csf[0:1, :],
                    onesp2[:CPB, 0, :],
                    nt_all[:CPB, j, :],
                    start=(j == 0),
                    stop=(j == NB - 1),
                )
        if p == 3:
            # counts . |c|^2 (norms^2 host-packed beside the counts)
            cscr = pnrm.tile([CPB, NB], _F32, tag="cscr")
            nc.vector.tensor_tensor(
                out=cscr[:],
                in0=cntt[:CPB, 0:NB],
                in1=cntt[:CPB, NB:16],
                op=mybir.AluOpType.mult,
            )
            nc.vector.tensor_reduce(
                out=staging[:CPB, 8:9],
                in_=cscr[:],
                axis=mybir.AxisListType.X,
                op=mybir.AluOpType.add,
            )

    pf = pG.tile([1, 32], _F32, tag="cs")
    nc.tensor.matmul(pf[:], ones[:], staging[:], start=True, stop=True)
    osb = const.tile([1, 32], _F32)
    nc.vector.tensor_copy(osb[:], pf[:])
    nc.sync.dma_start(out[:, :], osb[:, :])


def build():
    if "nc" in _NC_CACHE:
        return _NC_CACHE["nc"]
    nc = bacc.Bacc(
        "TRN2",
        target_bir_lowering=False,
        debug=False,
        enable_asserts=False,
        num_devices=N_CORES,
    )
    feat = nc.dram_tensor("feat", [PR, D], _FP8, kind="ExternalInput").ap()
    cnt = nc.dram_tensor("cnt", [128, 16], _F32, kind="ExternalInput").ap()
    oh = nc.dram_tensor("oh", [128, 4 * PSLOT, 128], _FP8, kind="ExternalInput").ap()
    cbf = nc.dram_tensor("ctab", [128 * NB, D], _FP8, kind="ExternalInput").ap()
    nbf = nc.dram_tensor("ntab", [128 * NB, D], _FP8, kind="ExternalInput").ap()
    out = nc.dram_tensor("out", [1, 32], _F32, kind="ExternalOutput").ap()
    with tile.TileContext(nc) as tc, ExitStack() as ctx:
        _build_body(ctx, tc, feat, cnt, oh, cbf, nbf, out)
    nc.compile()
    _NC_CACHE["nc"] = nc
    return nc


def _bank_assignment(y):
    """Greedy-balanced partition of the C classes into NB banks of CPB each."""
    counts = np.bincount(y, minlength=C)
    order = np.argsort(-counts, kind="stable")
    bank_tot = np.zeros(NB, dtype=np.int64)
    bank_n = np.zeros(NB, dtype=np.int64)
    bankclasses = np.zeros((NB, CPB), dtype=np.int64)
    cls_bank = np.zeros(C, dtype=np.int64)
    cls_pos = np.zeros(C, dtype=np.int64)
    for c in order:
        open_banks = np.flatnonzero(bank_n < CPB)
        j = open_banks[np.argmin(bank_tot[open_banks])]
        bankclasses[j, bank_n[j]] = c
        cls_bank[c] = j
        cls_pos[c] = bank_n[j]
        bank_n[j] += 1
        bank_tot[j] += counts[c]
    assert bank_tot.max() <= 128 * SUB, f"bank overflow: {bank_tot.max()}"
    return bankclasses, cls_bank, cls_pos, counts


def make_in_maps(y, feat, centers):
    feat = np.ascontiguousarray(feat, dtype=np.float32)
    centers = np.ascontiguousarray(centers, dtype=np.float32)
    y = np.asarray(y).astype(np.int64)
    norm2 = np.sum(centers.astype(np.float64) ** 2, axis=1, keepdims=True)
    ncenters = (centers / np.sqrt(norm2)).astype(ml_dtypes.float8_e4m3)
    slot_of = np.zeros((2, SUB), dtype=np.int64)
    slot_of[0] = SLOT_A
    slot_of[1] = SLOT_B
    in_maps = []
    for i in range(N_CORES):
        ys = y[i * BS : (i + 1) * BS]
        fs = feat[i * BS : (i + 1) * BS]
        bankclasses, cls_bank, cls_pos, counts = _bank_assignment(ys)

        # bank-major padded tables: dram row r (r%128 < 125) = class
        # bankclasses[r // 128][r % 128]
        ctab = np.zeros((128 * NB, D), dtype=ml_dtypes.float8_e4m3)
        ntab = np.zeros((128 * NB, D), dtype=ml_dtypes.float8_e4m3)
        rr = np.arange(128 * NB)
        vr = rr % 128 < CPB
        src = bankclasses[rr[vr] // 128, rr[vr] % 128]
        ctab[vr] = centers[src].astype(ml_dtypes.float8_e4m3)
        if i == 0:
            ntab[vr] = ncenters[src]
        elif i == 1:
            # D-rotated so this core's Gram pass covers rows [256:512)
            ntab[vr] = ncenters[src][:, (np.arange(D) + 256) % D]
        # cores 2-7 keep zero tables: their Gram/colsum outputs are zero

        # bucket rows by bank; bank q's i-th row sits at pair p=q//2,
        # partition i//9, slot slot_of[q%2][i%9]; padded row index is
        # p*PGROUP + 18*part + slot
        row_bank = cls_bank[ys]
        grp_order = np.argsort(row_bank, kind="stable")
        n_per = np.bincount(row_bank, minlength=NB)
        starts = np.zeros(NB + 1, dtype=np.int64)
        starts[1:] = np.cumsum(n_per)

        featp = np.zeros((PR, D), dtype=ml_dtypes.float8_e4m3)
        oh = np.zeros((128, 4 * PSLOT, 128), dtype=ml_dtypes.float8_e4m3)
        for q in range(NB):
            rows = grp_order[starts[q] : starts[q + 1]]
            idx = np.arange(len(rows))
            part = idx // SUB
            slot = slot_of[q % 2][idx % SUB]
            prow = (q // 2) * PGROUP + 18 * part + slot
            featp[prow] = fs[rows].astype(ml_dtypes.float8_e4m3)
            oh[part, (q // 2) * PSLOT + slot, cls_pos[ys[rows]]] = 1.0

        cnt_pb = np.zeros((128, 16), dtype=np.float32)
        cnt_pb[cls_pos, cls_bank] = counts
        cnt_pb[cls_pos, NB + cls_bank] = norm2[:, 0]

        in_maps.append(
            {
                "feat": featp,
                "cnt": cnt_pb,
                "oh": oh,
                "ctab": ctab,
                "ntab": ntab,
            }
        )
    return in_maps


def combine(outs):
    """outs: list of 8 [1,32] f32 arrays -> scalar loss (np.float32)."""
    cen = 0.0
    for o in outs:
        o = np.asarray(o, dtype=np.float64)
        cen += o[0, 0:9].sum() + o[0, 16:24].sum() - 2.0 * o[0, 10:14].sum()
    o0 = np.asarray(outs[0], dtype=np.float64)
    o1 = np.asarray(outs[1], dtype=np.float64)
    gsq, ssq = o0[0, 9] + o1[0, 9], o0[0, 14]
    ang = gsq - 2.0 * CT * ssq + C * C * CT * CT - C * (1.0 - CT) ** 2
    loss = 0.5 * cen / B + ang / (0.5 * C * (C - 1))
    return np.float32(loss)


def kernel(y, feat, centers):
    nc = build()
    in_maps = make_in_maps(y, feat, centers)
    res = run_bass_kernel_spmd(nc, in_maps, core_ids=list(range(N_CORES)))
    return combine([res.results[i]["out"] for i in range(N_CORES)])
